# revision 6
# baseline (speedup 1.0000x reference)
"""Causal self-attention Trainium2 kernel (B=128, T=128, C=768, H=12, D=64).

Sharding: data-parallel over batch across 8 cores (16 batches/core).
Per-core pipeline (4-batch groups, feature-major activations):
  x -> PE-transpose -> x^T (fp32r)
  Q^T,K^T = W_qkv^T @ x^T  (fp32r matmuls, N=512)  -> fp16 tiles (K^T zero-padded)
  V       = x @ W_qkv[:,v] (token-major, fp32r, N=384) -> fp16 V' with ones col
  S^T_h   = Kz_h^T.T @ Q^T (fp16, K=128; +mask add via matmul)
  E^T     = exp(S^T * scale)  (ScalarE, fp16 out)
  O'_h    = E^T_h.T @ V'_h  (fp16, N=65: cols 0:64=O', col 64=rowsum)
  O       = O' * (1/rowsum)  (per-partition scalar, token-major, fp16)
  O^T via xbar DMA-transpose -> Y = O^T.T @ W_proj16 (fp16, N=384) -> DRAM
"""
import sys
import numpy as np

sys.path.insert(0, "/opt/trn_rl_repo")

import concourse.bass as bass  # noqa: E402
import concourse.tile as tile  # noqa: E402
from concourse import bacc, mybir  # noqa: E402
from concourse import bass_utils  # noqa: E402
from contextlib import ExitStack, nullcontext  # noqa: E402

F32 = mybir.dt.float32
F32R = mybir.dt.float32r
F16 = mybir.dt.float16

N_CORES = 8
B, T, C = 128, 128, 768
H, D = 12, 64
BC = B // N_CORES          # batches per core = 16
GB = 4                     # batches per group
NG = BC // GB              # groups per core = 4
GT = GB * T                # tokens per group = 512
NK = C // 128              # contraction k-tiles = 6
SCALE = D ** -0.5
MASKVAL = -30000.0


def build_program(loop_iters=None):
    nc = bacc.Bacc("TRN2", target_bir_lowering=False, debug=False,
                   num_devices=N_CORES)
    x_d = nc.dram_tensor("x", [BC, T, C], F32, kind="ExternalInput").ap()
    wqkv_d = nc.dram_tensor("w_qkv", [C, 3 * C], F32, kind="ExternalInput").ap()
    wproj16_d = nc.dram_tensor("w_proj16", [C, C], F16, kind="ExternalInput").ap()
    mask_d = nc.dram_tensor("mask16", [128, T], F16, kind="ExternalInput").ap()
    ident16_d = nc.dram_tensor("ident16", [128, 128], F16, kind="ExternalInput").ap()
    ident32_d = nc.dram_tensor("ident32", [128, 128], F32, kind="ExternalInput").ap()
    y_d = nc.dram_tensor("y", [BC, T, C], F32, kind="ExternalOutput").ap()

    with tile.TileContext(nc) as tc, ExitStack() as ctx:
        cpool = ctx.enter_context(tc.tile_pool(name="const", bufs=1))
        gpool = ctx.enter_context(tc.tile_pool(name="grp", bufs=2))
        spool = ctx.enter_context(tc.tile_pool(name="small", bufs=4))
        pp = ctx.enter_context(tc.tile_pool(name="ps", bufs=1, space="PSUM"))

        # ---- constants / weights (resident) ----
        wqkv_r = cpool.tile([128, NK, 3 * C], F32R)
        nc.sync.dma_start(wqkv_r, wqkv_d.rearrange("(k p) f -> p k f", p=128).bitcast(F32R))
        wproj16 = cpool.tile([128, NK, C], F16)
        nc.sync.dma_start(wproj16, wproj16_d.rearrange("(k p) f -> p k f", p=128))
        mask16 = cpool.tile([128, T], F16)
        nc.sync.dma_start(mask16, mask_d)
        ident16 = cpool.tile([128, 128], F16)
        nc.sync.dma_start(ident16, ident16_d)
        ident32 = cpool.tile([128, 128], F32)
        nc.sync.dma_start(ident32, ident32_d)
        ii16 = ident16[:, None, :].broadcast_to([128, 2, 128])

        # persistent kz / vp (ping-pong): zero halves and ones cols written once
        kz_pp = [cpool.tile([128, H, GT], F16, name=f"kz{i}") for i in range(2)]
        vp_pp = [cpool.tile([128, GB, H, 65], F16, name=f"vp{i}") for i in range(2)]
        for kzt in kz_pp:
            nc.gpsimd.memset(kzt[64:128, 0:H:2, :], 0.0)
            nc.gpsimd.memset(kzt[0:64, 1:H:2, :], 0.0)
        for vpt in vp_pp:
            nc.gpsimd.memset(vpt[:, :, :, 64:65], 1.0)

        loop_cm = tc.For_i(0, loop_iters, 1) if loop_iters else nullcontext()
        with loop_cm:
            for g in range(NG):
                # ---- load x group [t, b, c] ----
                x_sb = gpool.tile([128, GB, C], F32, tag="x_sb", bufs=1, name=f"x_sb_{g}")
                nc.sync.dma_start(x_sb, x_d[g * GB:(g + 1) * GB].rearrange("b t c -> t b c"))

                # ---- x^T via PE transpose ----
                xT = gpool.tile([128, NK, GB, 128], F32R, tag="xT", name=f"xT_{g}")
                for b in range(GB):
                    for k in range(NK):
                        xt_ps = pp.tile([128, 128], F32, tag="tps", bufs=1, name="xt_ps")
                        nc.tensor.transpose(xt_ps, x_sb[:, b, 128 * k:128 * (k + 1)], ident32)
                        nc.vector.tensor_copy(xT[:, k, b, :], xt_ps)

                # ---- Q^T / K^T projection (feature-major, fp32r, N=512) ----
                qT = gpool.tile([128, 6, GT], F16, tag="qT", name=f"qT_{g}")
                kz = kz_pp[g % 2]
                xg = xT.rearrange("p k b t -> p k (b t)")
                for f in range(12):
                    qk_ps = pp.tile([128, GT], F32, tag="qkps", bufs=2, name="qk_ps")
                    for k in range(NK):
                        nc.tensor.matmul(qk_ps, wqkv_r[:, k, 128 * f:128 * (f + 1)],
                                         xg[:, k, :], start=(k == 0), stop=(k == NK - 1))
                    if f < 6:
                        nc.scalar.copy(qT[:, f, :], qk_ps)
                    else:
                        h0 = 2 * (f - 6)
                        nc.vector.tensor_copy(kz[0:64, h0, :], qk_ps[0:64, :])
                        nc.vector.tensor_copy(kz[64:128, h0 + 1, :], qk_ps[64:128, :])

                # ---- V projection (token-major, fp32r, N=384) into V' ----
                vp = vp_pp[g % 2]
                for b in range(GB):
                    for half in range(2):
                        v_ps = pp.tile([128, 384], F32, tag="vps", bufs=2, name="v_ps")
                        for k in range(NK):
                            nc.tensor.matmul(
                                v_ps, xT[:, k, b, :],
                                wqkv_r[:, k, 2 * C + 384 * half:2 * C + 384 * (half + 1)],
                                start=(k == 0), stop=(k == NK - 1))
                        nc.scalar.copy(
                            vp[:, b, 6 * half:6 * (half + 1), 0:64],
                            v_ps.rearrange("p (h d) -> p h d", d=64))

                # ---- attention (token-major O, fp16) ----
                o_sb = gpool.tile([128, GB, C], F16, tag="o_sb", name=f"o_sb_{g}")
                for b in range(GB):
                    for pr in range(6):
                        h0 = 2 * pr
                        qs = qT[:, pr, b * T:(b + 1) * T]
                        st_ps = pp.tile([128, 2 * T], F32, tag="att", bufs=3, name="st_ps")
                        nc.tensor.matmul(st_ps[:, 0:T], kz[:, h0, b * T:(b + 1) * T], qs,
                                         start=True, stop=False)
                        nc.tensor.matmul(st_ps[:, T:2 * T], kz[:, h0 + 1, b * T:(b + 1) * T],
                                         qs, start=False, stop=False)
                        nc.tensor.matmul(st_ps, mask16, ii16, start=False, stop=True)
                        eT = spool.tile([128, 2 * T], F16, tag="eT", name="eT")
                        nc.scalar.activation(eT, st_ps, mybir.ActivationFunctionType.Exp,
                                             scale=SCALE)
                        for hh in range(2):
                            h = h0 + hh
                            op_ps = pp.tile([128, 65], F32, tag="att", bufs=3, name="op_ps")
                            nc.tensor.matmul(op_ps, eT[:, hh * T:(hh + 1) * T],
                                             vp[:, b, h, :], start=True, stop=True)
                            rinv = spool.tile([128, 1], F32, tag="rinv", name="rinv")
                            nc.vector.reciprocal(rinv, op_ps[:, 64:65])
                            nc.vector.tensor_scalar_mul(
                                o_sb[:, b, h * D:(h + 1) * D], op_ps[:, 0:64], rinv)

                # ---- O^T via xbar DMA transpose, then Y projection (fp16) ----
                for b in range(GB):
                    oT = spool.tile([128, NK, 128], F16, tag="oT", bufs=2, name="oT")
                    nc.sync.dma_start_transpose(oT, o_sb[:, b, :])
                    y_sb = spool.tile([128, C], F32, tag="y_sb", bufs=2, name="y_sb")
                    for half in range(2):
                        y_ps = pp.tile([128, 384], F32, tag="vps", bufs=2, name="y_ps")
                        for k in range(NK):
                            nc.tensor.matmul(y_ps, oT[:, k, :],
                                             wproj16[:, k, 384 * half:384 * (half + 1)],
                                             start=(k == 0), stop=(k == NK - 1))
                        nc.scalar.copy(y_sb[:, 384 * half:384 * (half + 1)], y_ps)
                    nc.sync.dma_start(y_d[g * GB + b], y_sb)

    nc.compile()
    return nc


_PROGRAM = None
_in_maps_cache = None


def _host_consts():
    mask16 = np.where(np.arange(T)[None, :] <= np.arange(128)[:, None],
                      0.0, MASKVAL).astype(np.float16)
    ident16 = np.eye(128, dtype=np.float16)
    ident32 = np.eye(128, dtype=np.float32)
    return mask16, ident16, ident32


def make_in_maps(x, w_qkv, w_proj):
    x = np.ascontiguousarray(np.asarray(x), dtype=np.float32)
    w_qkv = np.ascontiguousarray(np.asarray(w_qkv), dtype=np.float32)
    w_proj16 = np.ascontiguousarray(np.asarray(w_proj), dtype=np.float16)
    mask16, ident16, ident32 = _host_consts()
    in_maps = []
    for c in range(N_CORES):
        in_maps.append({
            "x": x[c * BC:(c + 1) * BC],
            "w_qkv": w_qkv,
            "w_proj16": w_proj16,
            "mask16": mask16,
            "ident16": ident16,
            "ident32": ident32,
        })
    return in_maps


def kernel(x, w_qkv, w_proj):
    global _PROGRAM, _in_maps_cache
    if _PROGRAM is None:
        _PROGRAM = build_program()
    nc = _PROGRAM
    in_maps = make_in_maps(x, w_qkv, w_proj)
    _in_maps_cache = in_maps
    res = bass_utils.run_bass_kernel_spmd(nc, in_maps, core_ids=list(range(N_CORES)))
    out = np.concatenate([r["y"] for r in res.results], axis=0)
    return out.astype(np.float32)


# revision 7
# speedup vs baseline: 1.3072x; 1.3072x over previous
"""Causal self-attention Trainium2 kernel (B=128, T=128, C=768, H=12, D=64).

Sharding: data-parallel over batch across 8 cores (16 batches/core).
Per-core pipeline (4-batch groups, feature-major activations):
  x -> PE-transpose -> x^T (fp32r)
  Q^T,K^T = W_qkv^T @ x^T  (fp32r matmuls, N=512)  -> fp16 tiles (K^T zero-padded)
  V       = x @ W_qkv[:,v] (token-major, fp32r, N=384) -> fp16 V' with ones col
  S^T_h   = Kz_h^T.T @ Q^T (fp16, K=128; +mask add via matmul)
  E^T     = exp(S^T * scale)  (ScalarE, fp16 out)
  O'_h    = E^T_h.T @ V'_h  (fp16, N=65: cols 0:64=O', col 64=rowsum)
  O       = O' * (1/rowsum)  (per-partition scalar, token-major, fp16)
  O^T via xbar DMA-transpose -> Y = O^T.T @ W_proj16 (fp16, N=384) -> DRAM
"""
import sys
import numpy as np

sys.path.insert(0, "/opt/trn_rl_repo")

import concourse.bass as bass  # noqa: E402
import concourse.tile as tile  # noqa: E402
from concourse import bacc, mybir  # noqa: E402
from concourse import bass_utils  # noqa: E402
from contextlib import ExitStack, nullcontext  # noqa: E402

F32 = mybir.dt.float32
F32R = mybir.dt.float32r
F16 = mybir.dt.float16

N_CORES = 8
B, T, C = 128, 128, 768
H, D = 12, 64
BC = B // N_CORES          # batches per core = 16
GB = 4                     # batches per group
NG = BC // GB              # groups per core = 4
GT = GB * T                # tokens per group = 512
NK = C // 128              # contraction k-tiles = 6
SCALE = D ** -0.5
MASKVAL = -30000.0


def build_program(loop_iters=None):
    nc = bacc.Bacc("TRN2", target_bir_lowering=False, debug=False,
                   num_devices=N_CORES)
    x_d = nc.dram_tensor("x", [BC, T, C], F32, kind="ExternalInput").ap()
    wqkv_d = nc.dram_tensor("w_qkv", [C, 3 * C], F32, kind="ExternalInput").ap()
    wproj16_d = nc.dram_tensor("w_proj16", [C, C], F16, kind="ExternalInput").ap()
    mask_d = nc.dram_tensor("mask16", [128, T], F16, kind="ExternalInput").ap()
    ident16_d = nc.dram_tensor("ident16", [128, 128], F16, kind="ExternalInput").ap()
    ident32_d = nc.dram_tensor("ident32", [128, 128], F32, kind="ExternalInput").ap()
    y_d = nc.dram_tensor("y", [BC, T, C], F32, kind="ExternalOutput").ap()

    with tile.TileContext(nc) as tc, ExitStack() as ctx:
        cpool = ctx.enter_context(tc.tile_pool(name="const", bufs=1))
        gpool = ctx.enter_context(tc.tile_pool(name="grp", bufs=2))
        spool = ctx.enter_context(tc.tile_pool(name="small", bufs=4))
        pp = ctx.enter_context(tc.tile_pool(name="ps", bufs=1, space="PSUM"))

        # ---- constants / weights (resident) ----
        wqkv_r = cpool.tile([128, NK, 3 * C], F32R)
        nc.gpsimd.dma_start(wqkv_r, wqkv_d.rearrange("(k p) f -> p k f", p=128).bitcast(F32R))
        wproj16 = cpool.tile([128, NK, C], F16)
        nc.gpsimd.dma_start(wproj16, wproj16_d.rearrange("(k p) f -> p k f", p=128))
        mask16 = cpool.tile([128, T], F16)
        nc.sync.dma_start(mask16, mask_d)
        ident16 = cpool.tile([128, 128], F16)
        nc.sync.dma_start(ident16, ident16_d)
        ident32 = cpool.tile([128, 128], F32)
        nc.sync.dma_start(ident32, ident32_d)
        ii16 = ident16[:, None, :].broadcast_to([128, 2, 128])

        # persistent kz / vp (ping-pong): zero halves and ones cols written once
        kz_pp = [cpool.tile([128, H, GT], F16, name=f"kz{i}") for i in range(2)]
        vp_pp = [cpool.tile([128, GB, H, 65], F16, name=f"vp{i}") for i in range(2)]
        for kzt in kz_pp:
            nc.gpsimd.memset(kzt[64:128, 0:H:2, :], 0.0)
            nc.gpsimd.memset(kzt[0:64, 1:H:2, :], 0.0)
        for vpt in vp_pp:
            nc.gpsimd.memset(vpt[:, :, :, 64:65], 1.0)

        loop_cm = tc.For_i(0, loop_iters, 1) if loop_iters else nullcontext()
        with loop_cm:
            for g in range(NG):
                # ---- load x per batch + x^T via PE transpose ----
                xT = gpool.tile([128, NK, GB, 128], F32R, tag="xT", name=f"xT_{g}")
                for b in range(GB):
                    x_sb = gpool.tile([128, C], F32, tag="x_sb", bufs=3, name=f"x_sb_{g}_{b}")
                    nc.sync.dma_start(x_sb, x_d[g * GB + b].rearrange("t c -> t c"))
                    for k in range(NK):
                        xt_ps = pp.tile([128, 128], F32, tag="tps", bufs=1, name="xt_ps")
                        nc.tensor.transpose(xt_ps, x_sb[:, 128 * k:128 * (k + 1)], ident32)
                        nc.vector.tensor_copy(xT[:, k, b, :], xt_ps)

                # ---- Q^T / K^T projection (feature-major, fp32r, N=512) ----
                qT = gpool.tile([128, 6, GT], F16, tag="qT", name=f"qT_{g}")
                kz = kz_pp[g % 2]
                xg = xT.rearrange("p k b t -> p k (b t)")
                for f in range(12):
                    qk_ps = pp.tile([128, GT], F32, tag="qkps", bufs=2, name="qk_ps")
                    for k in range(NK):
                        nc.tensor.matmul(qk_ps, wqkv_r[:, k, 128 * f:128 * (f + 1)],
                                         xg[:, k, :], start=(k == 0), stop=(k == NK - 1))
                    if f < 6:
                        nc.scalar.copy(qT[:, f, :], qk_ps)
                    else:
                        h0 = 2 * (f - 6)
                        nc.vector.tensor_copy(kz[0:64, h0, :], qk_ps[0:64, :])
                        nc.vector.tensor_copy(kz[64:128, h0 + 1, :], qk_ps[64:128, :])

                # ---- V projection (token-major, fp32r, N=384) into V' ----
                vp = vp_pp[g % 2]
                for b in range(GB):
                    for half in range(2):
                        v_ps = pp.tile([128, 384], F32, tag="vps", bufs=2, name="v_ps")
                        for k in range(NK):
                            nc.tensor.matmul(
                                v_ps, xT[:, k, b, :],
                                wqkv_r[:, k, 2 * C + 384 * half:2 * C + 384 * (half + 1)],
                                start=(k == 0), stop=(k == NK - 1))
                        nc.scalar.copy(
                            vp[:, b, 6 * half:6 * (half + 1), 0:64],
                            v_ps.rearrange("p (h d) -> p h d", d=64))

                # ---- attention (token-major O, fp16) ----
                o_sb = gpool.tile([128, GB, C], F16, tag="o_sb", name=f"o_sb_{g}")
                for b in range(GB):
                    for pr in range(6):
                        h0 = 2 * pr
                        qs = qT[:, pr, b * T:(b + 1) * T]
                        st_ps = pp.tile([128, 2 * T], F32, tag="att", bufs=3, name="st_ps")
                        nc.tensor.matmul(st_ps[:, 0:T], kz[:, h0, b * T:(b + 1) * T], qs,
                                         start=True, stop=False)
                        nc.tensor.matmul(st_ps[:, T:2 * T], kz[:, h0 + 1, b * T:(b + 1) * T],
                                         qs, start=False, stop=False)
                        nc.tensor.matmul(st_ps, mask16, ii16, start=False, stop=True)
                        eT = spool.tile([128, 2 * T], F16, tag="eT", name="eT")
                        nc.scalar.activation(eT, st_ps, mybir.ActivationFunctionType.Exp,
                                             scale=SCALE)
                        for hh in range(2):
                            h = h0 + hh
                            op_ps = pp.tile([128, 65], F32, tag="att", bufs=3, name="op_ps")
                            nc.tensor.matmul(op_ps, eT[:, hh * T:(hh + 1) * T],
                                             vp[:, b, h, :], start=True, stop=True)
                            rinv = spool.tile([128, 1], F32, tag="rinv", name="rinv")
                            nc.vector.reciprocal(rinv, op_ps[:, 64:65])
                            nc.vector.tensor_scalar_mul(
                                o_sb[:, b, h * D:(h + 1) * D], op_ps[:, 0:64], rinv)

                # ---- O^T via xbar DMA transpose, then Y projection (fp16) ----
                for b in range(GB):
                    oT = spool.tile([128, NK, 128], F16, tag="oT", bufs=2, name="oT")
                    nc.sync.dma_start_transpose(oT, o_sb[:, b, :])
                    y_sb = spool.tile([128, C], F32, tag="y_sb", bufs=2, name="y_sb")
                    for half in range(2):
                        y_ps = pp.tile([128, 384], F32, tag="vps", bufs=2, name="y_ps")
                        for k in range(NK):
                            nc.tensor.matmul(y_ps, oT[:, k, :],
                                             wproj16[:, k, 384 * half:384 * (half + 1)],
                                             start=(k == 0), stop=(k == NK - 1))
                        nc.scalar.copy(y_sb[:, 384 * half:384 * (half + 1)], y_ps)
                    nc.sync.dma_start(y_d[g * GB + b], y_sb)

    nc.compile()
    return nc


_PROGRAM = None
_in_maps_cache = None


def _host_consts():
    mask16 = np.where(np.arange(T)[None, :] <= np.arange(128)[:, None],
                      0.0, MASKVAL).astype(np.float16)
    ident16 = np.eye(128, dtype=np.float16)
    ident32 = np.eye(128, dtype=np.float32)
    return mask16, ident16, ident32


def make_in_maps(x, w_qkv, w_proj):
    x = np.ascontiguousarray(np.asarray(x), dtype=np.float32)
    w_qkv = np.ascontiguousarray(np.asarray(w_qkv), dtype=np.float32)
    w_proj16 = np.ascontiguousarray(np.asarray(w_proj), dtype=np.float16)
    mask16, ident16, ident32 = _host_consts()
    in_maps = []
    for c in range(N_CORES):
        in_maps.append({
            "x": x[c * BC:(c + 1) * BC],
            "w_qkv": w_qkv,
            "w_proj16": w_proj16,
            "mask16": mask16,
            "ident16": ident16,
            "ident32": ident32,
        })
    return in_maps


def kernel(x, w_qkv, w_proj):
    global _PROGRAM, _in_maps_cache
    if _PROGRAM is None:
        _PROGRAM = build_program()
    nc = _PROGRAM
    in_maps = make_in_maps(x, w_qkv, w_proj)
    _in_maps_cache = in_maps
    res = bass_utils.run_bass_kernel_spmd(nc, in_maps, core_ids=list(range(N_CORES)))
    out = np.concatenate([r["y"] for r in res.results], axis=0)
    return out.astype(np.float32)


# revision 8
# speedup vs baseline: 1.3562x; 1.0374x over previous
"""Causal self-attention Trainium2 kernel (B=128, T=128, C=768, H=12, D=64).

Sharding: data-parallel over batch across 8 cores (16 batches/core).
Per-core pipeline (4-batch groups, feature-major activations):
  x -> PE-transpose -> x^T (fp32r)
  Q^T,K^T = W_qkv^T @ x^T  (fp32r matmuls, N=512)  -> fp16 tiles (K^T zero-padded)
  V       = x @ W_qkv[:,v] (token-major, fp32r, N=384) -> fp16 V' with ones col
  S^T_h   = Kz_h^T.T @ Q^T (fp16, K=128; +mask add via matmul)
  E^T     = exp(S^T * scale)  (ScalarE, fp16 out)
  O'_h    = E^T_h.T @ V'_h  (fp16, N=65: cols 0:64=O', col 64=rowsum)
  O       = O' * (1/rowsum)  (per-partition scalar, token-major, fp16)
  O^T via xbar DMA-transpose -> Y = O^T.T @ W_proj16 (fp16, N=384) -> DRAM
"""
import sys
import numpy as np

sys.path.insert(0, "/opt/trn_rl_repo")

import concourse.bass as bass  # noqa: E402
import concourse.tile as tile  # noqa: E402
from concourse import bacc, mybir  # noqa: E402
from concourse import bass_utils  # noqa: E402
from contextlib import ExitStack, nullcontext  # noqa: E402

F32 = mybir.dt.float32
F32R = mybir.dt.float32r
F16 = mybir.dt.float16

N_CORES = 8
B, T, C = 128, 128, 768
H, D = 12, 64
BC = B // N_CORES          # batches per core = 16
GB = 4                     # batches per group
NG = BC // GB              # groups per core = 4
GT = GB * T                # tokens per group = 512
NK = C // 128              # contraction k-tiles = 6
SCALE = D ** -0.5
MASKVAL = -30000.0


def build_program(loop_iters=None):
    nc = bacc.Bacc("TRN2", target_bir_lowering=False, debug=False,
                   num_devices=N_CORES)
    x_d = nc.dram_tensor("x", [BC, T, C], F32, kind="ExternalInput").ap()
    wqkv_d = nc.dram_tensor("w_qkv", [C, 3 * C], F32, kind="ExternalInput").ap()
    wproj16_d = nc.dram_tensor("w_proj16", [C, C], F16, kind="ExternalInput").ap()
    mask_d = nc.dram_tensor("mask16", [128, T], F16, kind="ExternalInput").ap()
    ident16_d = nc.dram_tensor("ident16", [128, 128], F16, kind="ExternalInput").ap()
    ident32_d = nc.dram_tensor("ident32", [128, 128], F32, kind="ExternalInput").ap()
    y_d = nc.dram_tensor("y", [BC, T, C], F32, kind="ExternalOutput").ap()

    with tile.TileContext(nc) as tc, ExitStack() as ctx:
        cpool = ctx.enter_context(tc.tile_pool(name="const", bufs=1))
        gpool = ctx.enter_context(tc.tile_pool(name="grp", bufs=2))
        spool = ctx.enter_context(tc.tile_pool(name="small", bufs=4))
        pp = ctx.enter_context(tc.tile_pool(name="ps", bufs=1, space="PSUM"))

        # ---- constants / weights (resident) ----
        wqkv_r = cpool.tile([128, NK, 3 * C], F32R)
        for k in range(NK):
            nc.gpsimd.dma_start(wqkv_r[:, k, :],
                                wqkv_d.rearrange("(k p) f -> p k f", p=128)[:, k, :].bitcast(F32R))
        wproj16 = cpool.tile([128, NK, C], F16)
        nc.gpsimd.dma_start(wproj16, wproj16_d.rearrange("(k p) f -> p k f", p=128))
        mask16 = cpool.tile([128, T], F16)
        nc.sync.dma_start(mask16, mask_d)
        ident16 = cpool.tile([128, 128], F16)
        nc.sync.dma_start(ident16, ident16_d)
        ident32 = cpool.tile([128, 128], F32)
        nc.sync.dma_start(ident32, ident32_d)
        ii16 = ident16[:, None, :].broadcast_to([128, 2, 128])

        # persistent kz / vp (ping-pong): zero halves and ones cols written once
        kz_pp = [cpool.tile([128, H, GT], F16, name=f"kz{i}") for i in range(2)]
        vp_pp = [cpool.tile([128, GB, H, 65], F16, name=f"vp{i}") for i in range(2)]
        for kzt in kz_pp:
            nc.gpsimd.memset(kzt[64:128, 0:H:2, :], 0.0)
            nc.gpsimd.memset(kzt[0:64, 1:H:2, :], 0.0)
        for vpt in vp_pp:
            nc.gpsimd.memset(vpt[:, :, :, 64:65], 1.0)

        loop_cm = tc.For_i(0, loop_iters, 1) if loop_iters else nullcontext()
        with loop_cm:
            for g in range(NG):
                # ---- load x per batch + x^T via PE transpose ----
                xT = gpool.tile([128, NK, GB, 128], F32R, tag="xT", name=f"xT_{g}")
                for b in range(GB):
                    x_sb = gpool.tile([128, C], F32, tag="x_sb", bufs=3, name=f"x_sb_{g}_{b}")
                    nc.sync.dma_start(x_sb, x_d[g * GB + b].rearrange("t c -> t c"))
                    for k in range(NK):
                        xt_ps = pp.tile([128, 128], F32, tag="tps", bufs=1, name="xt_ps")
                        nc.tensor.transpose(xt_ps, x_sb[:, 128 * k:128 * (k + 1)], ident32)
                        nc.vector.tensor_copy(xT[:, k, b, :], xt_ps)

                # ---- Q^T / K^T projection (feature-major, fp32r, N=512) ----
                qT = gpool.tile([128, 6, GT], F16, tag="qT", name=f"qT_{g}")
                kz = kz_pp[g % 2]
                xg = xT.rearrange("p k b t -> p k (b t)")
                for f in range(12):
                    qk_ps = pp.tile([128, GT], F32, tag="qkps", bufs=2, name="qk_ps")
                    for k in range(NK):
                        nc.tensor.matmul(qk_ps, wqkv_r[:, k, 128 * f:128 * (f + 1)],
                                         xg[:, k, :], start=(k == 0), stop=(k == NK - 1))
                    if f < 6:
                        nc.scalar.copy(qT[:, f, :], qk_ps)
                    else:
                        h0 = 2 * (f - 6)
                        nc.vector.tensor_copy(kz[0:64, h0, :], qk_ps[0:64, :])
                        nc.vector.tensor_copy(kz[64:128, h0 + 1, :], qk_ps[64:128, :])

                # ---- V projection (token-major, fp32r, N=384) into V' ----
                vp = vp_pp[g % 2]
                for b in range(GB):
                    for half in range(2):
                        v_ps = pp.tile([128, 384], F32, tag="vps", bufs=2, name="v_ps")
                        for k in range(NK):
                            nc.tensor.matmul(
                                v_ps, xT[:, k, b, :],
                                wqkv_r[:, k, 2 * C + 384 * half:2 * C + 384 * (half + 1)],
                                start=(k == 0), stop=(k == NK - 1))
                        nc.scalar.copy(
                            vp[:, b, 6 * half:6 * (half + 1), 0:64],
                            v_ps.rearrange("p (h d) -> p h d", d=64))

                # ---- attention (token-major O, fp16) ----
                o_sb = gpool.tile([128, GB, C], F16, tag="o_sb", name=f"o_sb_{g}")
                for b in range(GB):
                    for pr in range(6):
                        h0 = 2 * pr
                        qs = qT[:, pr, b * T:(b + 1) * T]
                        st_ps = pp.tile([128, 2 * T], F32, tag="att", bufs=3, name="st_ps")
                        nc.tensor.matmul(st_ps[:, 0:T], kz[:, h0, b * T:(b + 1) * T], qs,
                                         start=True, stop=False)
                        nc.tensor.matmul(st_ps[:, T:2 * T], kz[:, h0 + 1, b * T:(b + 1) * T],
                                         qs, start=False, stop=False)
                        nc.tensor.matmul(st_ps, mask16, ii16, start=False, stop=True)
                        eT = spool.tile([128, 2 * T], F16, tag="eT", name="eT")
                        nc.scalar.activation(eT, st_ps, mybir.ActivationFunctionType.Exp,
                                             scale=SCALE)
                        for hh in range(2):
                            h = h0 + hh
                            op_ps = pp.tile([128, 65], F32, tag="att", bufs=3, name="op_ps")
                            nc.tensor.matmul(op_ps, eT[:, hh * T:(hh + 1) * T],
                                             vp[:, b, h, :], start=True, stop=True)
                            rinv = spool.tile([128, 1], F32, tag="rinv", name="rinv")
                            nc.vector.reciprocal(rinv, op_ps[:, 64:65])
                            nc.vector.tensor_scalar_mul(
                                o_sb[:, b, h * D:(h + 1) * D], op_ps[:, 0:64], rinv)

                # ---- O^T via xbar DMA transpose, then Y projection (fp16) ----
                for b in range(GB):
                    oT = spool.tile([128, NK, 128], F16, tag="oT", bufs=2, name="oT")
                    nc.sync.dma_start_transpose(oT, o_sb[:, b, :])
                    y_sb = spool.tile([128, C], F32, tag="y_sb", bufs=2, name="y_sb")
                    for half in range(2):
                        y_ps = pp.tile([128, 384], F32, tag="vps", bufs=2, name="y_ps")
                        for k in range(NK):
                            nc.tensor.matmul(y_ps, oT[:, k, :],
                                             wproj16[:, k, 384 * half:384 * (half + 1)],
                                             start=(k == 0), stop=(k == NK - 1))
                        nc.scalar.copy(y_sb[:, 384 * half:384 * (half + 1)], y_ps)
                    nc.sync.dma_start(y_d[g * GB + b], y_sb)

    nc.compile()
    return nc


_PROGRAM = None
_in_maps_cache = None


def _host_consts():
    mask16 = np.where(np.arange(T)[None, :] <= np.arange(128)[:, None],
                      0.0, MASKVAL).astype(np.float16)
    ident16 = np.eye(128, dtype=np.float16)
    ident32 = np.eye(128, dtype=np.float32)
    return mask16, ident16, ident32


def make_in_maps(x, w_qkv, w_proj):
    x = np.ascontiguousarray(np.asarray(x), dtype=np.float32)
    w_qkv = np.ascontiguousarray(np.asarray(w_qkv), dtype=np.float32)
    w_proj16 = np.ascontiguousarray(np.asarray(w_proj), dtype=np.float16)
    mask16, ident16, ident32 = _host_consts()
    in_maps = []
    for c in range(N_CORES):
        in_maps.append({
            "x": x[c * BC:(c + 1) * BC],
            "w_qkv": w_qkv,
            "w_proj16": w_proj16,
            "mask16": mask16,
            "ident16": ident16,
            "ident32": ident32,
        })
    return in_maps


def kernel(x, w_qkv, w_proj):
    global _PROGRAM, _in_maps_cache
    if _PROGRAM is None:
        _PROGRAM = build_program()
    nc = _PROGRAM
    in_maps = make_in_maps(x, w_qkv, w_proj)
    _in_maps_cache = in_maps
    res = bass_utils.run_bass_kernel_spmd(nc, in_maps, core_ids=list(range(N_CORES)))
    out = np.concatenate([r["y"] for r in res.results], axis=0)
    return out.astype(np.float32)
